# revision 8
# baseline (speedup 1.0000x reference)
"""CenterLoss kernel for Trainium2 (Bass/Tile), data-parallel over 8 NeuronCores.

reference:
    d_i = ||x_i||^2 + ||centers[l_i]||^2 - 2 x_i . centers[l_i]   (= ||x_i - c_{l_i}||^2)
    loss = mean_i clip(d_i, 1e-12, 1e12)

Only the label-gathered entry of the [N, C] distance matrix is used, so the
kernel never forms it: each core gathers centers[labels] with the Q7
dma_gather extended instruction, computes (x - c)^2 via DVE subtract + ACT
square-with-accumulate, reduces to a scalar partial sum, and the host
combines the 8 partials into the mean.  The clip is a no-op for this input
distribution (d_i concentrated around 256).

Performance structure (vs the naive version):
  * data moves at reduced precision (bf16 or fp8-e4m3); the mean absorbs the
    rounding noise (measured ~4e-6 rel for bf16, ~8e-4 for fp8, gate 2e-2).
  * rows are HOST-SORTED by label and grouped K to a gather index: one Q7
    descriptor then covers K rows (source = centers_rep, each row replicated
    K times).  SWDGE descriptor generation is the dominant wall (~9ns/idx per
    worker, only 4 generation streams), so K divides it.
  * class segments are padded to a multiple of K with zero-rows pointing at a
    zeros row of centers_rep -> pads contribute exactly 0 to the sum.
  * gathers are issued FIRST (background SWDGE queues 1-3, inline queue 0
    last) and x streams via a few large DMAs so the two transfer phases and
    compute fully overlap.

Sharding: x/labels split into 8 contiguous row shards; centers replicated.
"""

import numpy as np
import ml_dtypes

import concourse.bacc as bacc
import concourse.bass as bass
import concourse.tile as tile
from concourse import mybir
from concourse.bass_utils import run_bass_kernel_spmd
from concourse.library_config import mlp

N, C, D = 65536, 1000, 128
N_CORES = 8
P = 128
ROWS_PER_CORE = N // N_CORES            # 8192

# --- tunables -------------------------------------------------------------
DTYPE = "bf16"          # "bf16" | "fp8"
K = 1                   # rows per gather index (host sorts + pads classes)
NCHUNK = 8              # compute/gather chunks
X_DMAS = 4              # number of x dma_start instructions
SINGLE_PACKET = True
WARMUP = True
# queue per chunk; 1..3 = background SWDGE workers, 0 = inline on Pool engine
QUEUES = [1, 2, 3, 1, 2, 3, 0, 0]

_NP_DT = {"bf16": ml_dtypes.bfloat16, "fp8": ml_dtypes.float8_e4m3fn}
_MY_DT = {"bf16": mybir.dt.bfloat16, "fp8": mybir.dt.float8e4}

_NC_CACHE = {}


def _plan(labels_np):
    """Padded-rows plan shared by all cores (single SPMD program)."""
    if K == 1:
        rows_p = ROWS_PER_CORE
    else:
        need = 0
        for m in range(N_CORES):
            cnt = np.bincount(labels_np[m * ROWS_PER_CORE:(m + 1) * ROWS_PER_CORE],
                              minlength=C)
            need = max(need, ROWS_PER_CORE + int(((-cnt) % K).sum()))
        # rows per chunk must be a multiple of 128*K
        rows_p = -(-need // (NCHUNK * 128 * K)) * (NCHUNK * 128 * K)
    chunk = rows_p // NCHUNK
    return rows_p, chunk


def _build_nc(rows_p, chunk):
    f32 = mybir.dt.float32
    dt = _MY_DT[DTYPE]
    gc = chunk // K                 # gather indices per chunk
    s2 = gc // P                    # group slots per partition per chunk
    cols = chunk                    # elements per partition per chunk (s2*K*D/P*... = chunk)
    icols = gc // 16                # idx columns per chunk

    nc = bacc.Bacc(trn_type="TRN2", num_swdge_queues=4, dynamic_dma_scratch_size=65536)

    x = nc.dram_tensor("x", [NCHUNK, P, cols], dt, kind="ExternalInput")
    idx16 = nc.dram_tensor("idx16", [P, NCHUNK * icols], mybir.dt.int16,
                           kind="ExternalInput")
    centers_rep = nc.dram_tensor("centers_rep", [C + 1, K * D], dt,
                                 kind="ExternalInput")
    out = nc.dram_tensor("out", [1, 1], f32, kind="ExternalOutput")

    with tile.TileContext(nc) as tc:
        with (
            tc.tile_pool(name="cp", bufs=NCHUNK) as cp,
            tc.tile_pool(name="small", bufs=1) as small,
            tc.tile_pool(name="psp", bufs=1, space="PSUM") as psp,
        ):
            # eager Q7 library load so the first gather doesn't stall on the
            # lazy IRAM code fetch
            nc.gpsimd.load_library(mlp)

            idx = small.tile([P, NCHUNK * icols], mybir.dt.int16)
            nc.sync.dma_start(out=idx[:], in_=idx16.ap())

            # warmup: a 16-idx gather on every queue launches the background
            # SWDGE workers / loads their ucode immediately (otherwise the
            # first real gather pays a ~13us spin-up)
            if WARMUP:
                idxw = small.tile([P, 1], mybir.dt.int16)
                nc.vector.memset(idxw[:], 0.0)
                for wq in (1, 2, 3, 0):
                    wt = small.tile([P, K * D], dt, name=f"warm{wq}")
                    nc.gpsimd.dma_gather(
                        wt[:].rearrange("p (s e) -> p s e", s=1),
                        centers_rep.ap(), idxw[:, 0:1], 16, 16, K * D,
                        queue_num=wq, single_packet=SINGLE_PACKET,
                    )

            acc = small.tile([P, NCHUNK], f32)
            xts = [None] * NCHUNK
            cts = [cp.tile([P, cols], dt, tag="ct", name=f"ct{c}")
                   for c in range(NCHUNK)]

            def emit_gather(c):
                nc.gpsimd.dma_gather(
                    cts[c][:].rearrange("p (s e) -> p s e", s=s2),
                    centers_rep.ap(),
                    idx[:, c * icols:(c + 1) * icols],
                    gc, gc, K * D,
                    queue_num=QUEUES[c],
                    single_packet=SINGLE_PACKET,
                )

            # background-queue gathers first (cheap ring enqueues), inline last
            for c in range(NCHUNK):
                if QUEUES[c] != 0:
                    emit_gather(c)

            # x loads: a few large DMAs (each dma_start costs ~600ns of Sync)
            per = NCHUNK // X_DMAS
            for g in range(X_DMAS):
                dst = small.tile([P, per * cols], dt, tag=f"xg{g}")
                nc.sync.dma_start(
                    out=dst[:].rearrange("p (c f) -> p c f", c=per),
                    in_=x.ap()[g * per:(g + 1) * per].rearrange("c p f -> p c f"),
                )
                for c in range(g * per, (g + 1) * per):
                    xts[c] = dst[:, (c - g * per) * cols:(c - g * per + 1) * cols]

            for c in range(NCHUNK):
                if QUEUES[c] == 0:
                    emit_gather(c)

            for c in range(NCHUNK):
                xt, ct = xts[c], cts[c]
                nc.vector.tensor_tensor(out=xt, in0=xt, in1=ct[:],
                                        op=mybir.AluOpType.subtract)
                nc.scalar.activation(
                    out=xt, in_=xt,
                    func=mybir.ActivationFunctionType.Square,
                    accum_out=acc[:, c:c + 1],
                )

            dsum = small.tile([P, 1], f32)
            nc.vector.tensor_reduce(out=dsum[:], in_=acc[:], axis=mybir.AxisListType.X,
                                    op=mybir.AluOpType.add)
            ones = small.tile([P, 1], f32)
            nc.vector.memset(ones[:], 1.0)
            ps = psp.tile([1, 1], f32)
            nc.tensor.matmul(out=ps[:], lhsT=ones[:], rhs=dsum[:], start=True, stop=True)
            res = small.tile([1, 1], f32)
            nc.vector.tensor_copy(out=res[:], in_=ps[:])
            nc.sync.dma_start(out=out.ap(), in_=res[:])

    nc.compile()
    return nc


def _get_nc(rows_p, chunk):
    key = (DTYPE, K, NCHUNK, X_DMAS, SINGLE_PACKET, tuple(QUEUES), rows_p)
    if key not in _NC_CACHE:
        _NC_CACHE[key] = _build_nc(rows_p, chunk)
    return _NC_CACHE[key]


def _core_inputs(x_core, lab_core, rows_p, chunk):
    """Sort rows by label, pad class segments to K, lay out x in device order
    and build the wrapped int16 gather indices."""
    np_dt = _NP_DT[DTYPE]
    gc = chunk // K
    s2 = gc // P
    icols = gc // 16
    total_g = rows_p // K

    order = np.argsort(lab_core, kind="stable")
    slab = lab_core[order]
    if K == 1:
        g_of_row = np.arange(ROWS_PER_CORE)
        g_class = slab.astype(np.int16)
        n_groups = ROWS_PER_CORE
    else:
        cnt = np.bincount(slab, minlength=C)
        pad = (-cnt) % K
        gcnt = (cnt + pad) // K
        n_groups = int(gcnt.sum())
        # class of each real group, in sorted-class order
        g_class_real = np.repeat(np.arange(C), gcnt).astype(np.int16)
        # group id of each sorted row: per-class base + within-class offset
        gbase = np.concatenate([[0], np.cumsum(gcnt)[:-1]])
        within = np.arange(ROWS_PER_CORE) - np.repeat(
            np.concatenate([[0], np.cumsum(cnt)[:-1]]), cnt)
        g_of_row = gbase[slab] * K + within  # padded-row index of each sorted row
        g_class = np.full(total_g, C, dtype=np.int16)
        g_class[:n_groups] = g_class_real

    # device x layout: group g -> chunk g//gc, partition g%P... (see gather
    # doc: gathered index i lands on partition i%128, slot i//128)
    x_dev = np.zeros((NCHUNK, P, s2, K, D), dtype=np_dt)
    g = g_of_row if K > 1 else np.arange(ROWS_PER_CORE)
    grp = g // K
    k_off = g % K
    c_i = grp // gc
    g_loc = grp % gc
    p_i = g_loc % P
    s_i = g_loc // P
    x_dev[c_i, p_i, s_i, k_off, :] = x_core[order].astype(np_dt)

    idx16 = np.zeros((16, NCHUNK * icols), dtype=np.int16)
    gg = np.arange(total_g)
    idx16[(gg % gc) % 16, (gg // gc) * icols + (gg % gc) // 16] = g_class
    return (x_dev.reshape(NCHUNK, P, chunk),
            np.ascontiguousarray(np.tile(idx16, (8, 1))))


def make_in_maps(x, labels, centers, rows_p, chunk):
    x = np.ascontiguousarray(np.asarray(x), dtype=np.float32)
    labels_np = np.asarray(labels).astype(np.int64)
    centers = np.asarray(centers).astype(np.float32)
    crep = np.zeros((C + 1, K * D), dtype=_NP_DT[DTYPE])
    crep[:C] = np.tile(centers, (1, K)).astype(_NP_DT[DTYPE])
    in_maps = []
    for m in range(N_CORES):
        lo = m * ROWS_PER_CORE
        x_dev, idx16 = _core_inputs(x[lo:lo + ROWS_PER_CORE],
                                    labels_np[lo:lo + ROWS_PER_CORE], rows_p, chunk)
        in_maps.append({"x": x_dev, "idx16": idx16, "centers_rep": crep})
    return in_maps


def run(x, labels, centers, **spmd_kwargs):
    """Run on the 8 NeuronCores; returns (loss, BassKernelResults)."""
    labels_np = np.asarray(labels).astype(np.int64)
    rows_p, chunk = _plan(labels_np)
    nc = _get_nc(rows_p, chunk)
    in_maps = make_in_maps(x, labels_np, centers, rows_p, chunk)
    res = run_bass_kernel_spmd(nc, in_maps, core_ids=list(range(N_CORES)), **spmd_kwargs)
    total = sum(float(r["out"][0, 0]) for r in res.results)
    return np.float32(total / N), res


def kernel(x, labels, centers):
    loss, _ = run(x, labels, centers)
    return loss


# revision 11
# speedup vs baseline: 1.1134x; 1.1134x over previous
"""CenterLoss kernel for Trainium2 (Bass/Tile), data-parallel over 8 NeuronCores.

reference:
    d_i = ||x_i||^2 + ||centers[l_i]||^2 - 2 x_i . centers[l_i]   (= ||x_i - c_{l_i}||^2)
    loss = mean_i clip(d_i, 1e-12, 1e12)

Only the label-gathered entry of the [N, C] distance matrix is used, so the
kernel never forms it: each core gathers centers[labels] with the Q7
dma_gather extended instruction, computes (x - c)^2 via DVE subtract + ACT
square-with-accumulate, reduces to a scalar partial sum, and the host
combines the 8 partials into the mean.  The clip is a no-op for this input
distribution (d_i concentrated around 256).

Performance structure (vs the naive version):
  * data moves at reduced precision (bf16 or fp8-e4m3); the mean absorbs the
    rounding noise (measured ~4e-6 rel for bf16, ~8e-4 for fp8, gate 2e-2).
  * rows are HOST-SORTED by label and grouped K to a gather index: one Q7
    descriptor then covers K rows (source = centers_rep, each row replicated
    K times).  SWDGE descriptor generation is the dominant wall (~9ns/idx per
    worker, only 4 generation streams), so K divides it.
  * class segments are padded to a multiple of K with zero-rows pointing at a
    zeros row of centers_rep -> pads contribute exactly 0 to the sum.
  * gathers are issued FIRST (background SWDGE queues 1-3, inline queue 0
    last) and x streams via a few large DMAs so the two transfer phases and
    compute fully overlap.

Sharding: x/labels split into 8 contiguous row shards; centers replicated.
"""

import numpy as np
import ml_dtypes

import concourse.bacc as bacc
import concourse.bass as bass
import concourse.tile as tile
from concourse import mybir
from concourse.bass_utils import run_bass_kernel_spmd
from concourse.library_config import mlp

N, C, D = 65536, 1000, 128
N_CORES = 8
P = 128
ROWS_PER_CORE = N // N_CORES            # 8192

# --- tunables -------------------------------------------------------------
DTYPE = "fp8"           # "bf16" | "fp8"
K = 4                   # rows per gather index (host sorts + pads classes)
NCHUNK = 4              # compute/gather chunks
X_DMAS = 2              # number of x dma_start instructions
SINGLE_PACKET = True
WARMUP = False
# queue per chunk; 1..3 = background SWDGE workers, 0 = inline on Pool engine
QUEUES = [1, 2, 3, 0]

_NP_DT = {"bf16": ml_dtypes.bfloat16, "fp8": ml_dtypes.float8_e4m3fn}
_MY_DT = {"bf16": mybir.dt.bfloat16, "fp8": mybir.dt.float8e4}

_NC_CACHE = {}


def _plan(labels_np):
    """Padded-rows plan shared by all cores (single SPMD program)."""
    if K == 1:
        rows_p = ROWS_PER_CORE
    else:
        need = 0
        for m in range(N_CORES):
            cnt = np.bincount(labels_np[m * ROWS_PER_CORE:(m + 1) * ROWS_PER_CORE],
                              minlength=C)
            need = max(need, ROWS_PER_CORE + int(((-cnt) % K).sum()))
        # rows per chunk must be a multiple of 128*K
        rows_p = -(-need // (NCHUNK * 128 * K)) * (NCHUNK * 128 * K)
    chunk = rows_p // NCHUNK
    return rows_p, chunk


def _build_nc(rows_p, chunk):
    f32 = mybir.dt.float32
    dt = _MY_DT[DTYPE]
    gc = chunk // K                 # gather indices per chunk
    s2 = gc // P                    # group slots per partition per chunk
    cols = chunk                    # elements per partition per chunk (s2*K*D/P*... = chunk)
    icols = gc // 16                # idx columns per chunk

    nc = bacc.Bacc(trn_type="TRN2", num_swdge_queues=4, dynamic_dma_scratch_size=65536)

    x = nc.dram_tensor("x", [NCHUNK, P, cols], dt, kind="ExternalInput")
    idx16 = nc.dram_tensor("idx16", [P, NCHUNK * icols], mybir.dt.int16,
                           kind="ExternalInput")
    centers_rep = nc.dram_tensor("centers_rep", [C + 1, K * D], dt,
                                 kind="ExternalInput")
    out = nc.dram_tensor("out", [1, 1], f32, kind="ExternalOutput")

    with tile.TileContext(nc) as tc:
        with (
            tc.tile_pool(name="cp", bufs=NCHUNK) as cp,
            tc.tile_pool(name="small", bufs=1) as small,
            tc.tile_pool(name="psp", bufs=1, space="PSUM") as psp,
        ):
            # eager Q7 library load so the first gather doesn't stall on the
            # lazy IRAM code fetch
            nc.gpsimd.load_library(mlp)

            idx = small.tile([P, NCHUNK * icols], mybir.dt.int16)
            nc.sync.dma_start(out=idx[:], in_=idx16.ap())

            # warmup: a 16-idx gather on every queue launches the background
            # SWDGE workers / loads their ucode immediately (otherwise the
            # first real gather pays a ~13us spin-up)
            if WARMUP:
                idxw = small.tile([P, 1], mybir.dt.int16)
                nc.vector.memset(idxw[:], 0.0)
                for wq in (1, 2, 3, 0):
                    wt = small.tile([P, K * D], dt, name=f"warm{wq}")
                    nc.gpsimd.dma_gather(
                        wt[:].rearrange("p (s e) -> p s e", s=1),
                        centers_rep.ap(), idxw[:, 0:1], 16, 16, K * D,
                        queue_num=wq, single_packet=SINGLE_PACKET,
                    )

            acc = small.tile([P, NCHUNK], f32)
            xts = [None] * NCHUNK
            cts = [cp.tile([P, cols], dt, tag="ct", name=f"ct{c}")
                   for c in range(NCHUNK)]

            def emit_gather(c):
                nc.gpsimd.dma_gather(
                    cts[c][:].rearrange("p (s e) -> p s e", s=s2),
                    centers_rep.ap(),
                    idx[:, c * icols:(c + 1) * icols],
                    gc, gc, K * D,
                    queue_num=QUEUES[c],
                    single_packet=SINGLE_PACKET,
                )

            # background-queue gathers first (cheap ring enqueues), inline last
            for c in range(NCHUNK):
                if QUEUES[c] != 0:
                    emit_gather(c)

            # x loads: a few large DMAs (each dma_start costs ~600ns of Sync)
            per = NCHUNK // X_DMAS
            for g in range(X_DMAS):
                dst = small.tile([P, per * cols], dt, tag=f"xg{g}")
                nc.sync.dma_start(
                    out=dst[:].rearrange("p (c f) -> p c f", c=per),
                    in_=x.ap()[g * per:(g + 1) * per].rearrange("c p f -> p c f"),
                )
                for c in range(g * per, (g + 1) * per):
                    xts[c] = dst[:, (c - g * per) * cols:(c - g * per + 1) * cols]

            for c in range(NCHUNK):
                if QUEUES[c] == 0:
                    emit_gather(c)

            for c in range(NCHUNK):
                xt, ct = xts[c], cts[c]
                nc.vector.tensor_tensor(out=xt, in0=xt, in1=ct[:],
                                        op=mybir.AluOpType.subtract)
                nc.scalar.activation(
                    out=xt, in_=xt,
                    func=mybir.ActivationFunctionType.Square,
                    accum_out=acc[:, c:c + 1],
                )

            dsum = small.tile([P, 1], f32)
            nc.vector.tensor_reduce(out=dsum[:], in_=acc[:], axis=mybir.AxisListType.X,
                                    op=mybir.AluOpType.add)
            ones = small.tile([P, 1], f32)
            nc.vector.memset(ones[:], 1.0)
            ps = psp.tile([1, 1], f32)
            nc.tensor.matmul(out=ps[:], lhsT=ones[:], rhs=dsum[:], start=True, stop=True)
            res = small.tile([1, 1], f32)
            nc.vector.tensor_copy(out=res[:], in_=ps[:])
            nc.sync.dma_start(out=out.ap(), in_=res[:])

    nc.compile()
    return nc


def _get_nc(rows_p, chunk):
    key = (DTYPE, K, NCHUNK, X_DMAS, SINGLE_PACKET, tuple(QUEUES), rows_p)
    if key not in _NC_CACHE:
        _NC_CACHE[key] = _build_nc(rows_p, chunk)
    return _NC_CACHE[key]


def _core_inputs(x_core, lab_core, centers, rows_p, chunk):
    """Sort rows by label, pad class segments to K, lay out x in device order
    and build the wrapped int16 gather indices.  Pad slots inside a partial
    group are filled with that class's center so (x_pad - c)^2 == 0; filler
    groups beyond the last class point at the zeros row of centers_rep."""
    np_dt = _NP_DT[DTYPE]
    gc = chunk // K
    s2 = gc // P
    icols = gc // 16
    total_g = rows_p // K

    order = np.argsort(lab_core, kind="stable")
    slab = lab_core[order]
    vals = x_core[order]
    if K == 1:
        g_of_row = np.arange(ROWS_PER_CORE)
        g_class = slab.astype(np.int16)
        n_groups = ROWS_PER_CORE
    else:
        cnt = np.bincount(slab, minlength=C)
        pad = (-cnt) % K
        gcnt = (cnt + pad) // K
        n_groups = int(gcnt.sum())
        # class of each real group, in sorted-class order
        g_class_real = np.repeat(np.arange(C), gcnt).astype(np.int16)
        # padded-row index of each sorted row: per-class base + offset
        gbase = np.concatenate([[0], np.cumsum(gcnt)[:-1]])
        within = np.arange(ROWS_PER_CORE) - np.repeat(
            np.concatenate([[0], np.cumsum(cnt)[:-1]]), cnt)
        g_of_row = gbase[slab] * K + within
        g_class = np.full(total_g, C, dtype=np.int16)
        g_class[:n_groups] = g_class_real
        # pad slots of partial groups: x := c_j so the pad contributes 0
        pj = np.repeat(np.arange(C), pad)
        poff = np.arange(int(pad.sum())) - np.repeat(
            np.concatenate([[0], np.cumsum(pad)[:-1]]), pad)
        g_of_row = np.concatenate([g_of_row, gbase[pj] * K + cnt[pj] + poff])
        vals = np.concatenate([vals, centers[pj]])

    # device x layout: group g -> chunk g//gc, partition g%P, slot g//P (see
    # gather doc: gathered index i lands on partition i%128, slot i//128)
    x_dev = np.zeros((NCHUNK, P, s2, K, D), dtype=np_dt)
    g = g_of_row
    grp = g // K
    k_off = g % K
    c_i = grp // gc
    g_loc = grp % gc
    p_i = g_loc % P
    s_i = g_loc // P
    x_dev[c_i, p_i, s_i, k_off, :] = vals.astype(np_dt)

    idx16 = np.zeros((16, NCHUNK * icols), dtype=np.int16)
    gg = np.arange(total_g)
    idx16[(gg % gc) % 16, (gg // gc) * icols + (gg % gc) // 16] = g_class
    return (x_dev.reshape(NCHUNK, P, chunk),
            np.ascontiguousarray(np.tile(idx16, (8, 1))))


def make_in_maps(x, labels, centers, rows_p, chunk):
    x = np.ascontiguousarray(np.asarray(x), dtype=np.float32)
    labels_np = np.asarray(labels).astype(np.int64)
    centers = np.asarray(centers).astype(np.float32)
    crep = np.zeros((C + 1, K * D), dtype=_NP_DT[DTYPE])
    crep[:C] = np.tile(centers, (1, K)).astype(_NP_DT[DTYPE])
    in_maps = []
    for m in range(N_CORES):
        lo = m * ROWS_PER_CORE
        x_dev, idx16 = _core_inputs(x[lo:lo + ROWS_PER_CORE],
                                    labels_np[lo:lo + ROWS_PER_CORE], centers,
                                    rows_p, chunk)
        in_maps.append({"x": x_dev, "idx16": idx16, "centers_rep": crep})
    return in_maps


def run(x, labels, centers, **spmd_kwargs):
    """Run on the 8 NeuronCores; returns (loss, BassKernelResults)."""
    labels_np = np.asarray(labels).astype(np.int64)
    rows_p, chunk = _plan(labels_np)
    nc = _get_nc(rows_p, chunk)
    in_maps = make_in_maps(x, labels_np, centers, rows_p, chunk)
    res = run_bass_kernel_spmd(nc, in_maps, core_ids=list(range(N_CORES)), **spmd_kwargs)
    total = sum(float(r["out"][0, 0]) for r in res.results)
    return np.float32(total / N), res


def kernel(x, labels, centers):
    loss, _ = run(x, labels, centers)
    return loss


# revision 12
# speedup vs baseline: 1.1162x; 1.0025x over previous
"""CenterLoss kernel for Trainium2 (Bass/Tile), data-parallel over 8 NeuronCores.

reference:
    d_i = ||x_i||^2 + ||centers[l_i]||^2 - 2 x_i . centers[l_i]   (= ||x_i - c_{l_i}||^2)
    loss = mean_i clip(d_i, 1e-12, 1e12)

Only the label-gathered entry of the [N, C] distance matrix is used, so the
kernel never forms it: each core gathers centers[labels] with the Q7 dma_gather
extended instruction, computes (x - c)^2 (DVE subtract at bf16 2x + ACT
square-with-accumulate), and ships per-partition partial sums; the host
combines them into the mean.  The clip is a no-op for this input distribution
(d_i concentrated around 256; gate allows 2e-2 rel).

Performance structure:
  * rows are HOST-SORTED by label and grouped K=4 to a gather index; one Q7
    SWDGE descriptor covers 4 rows (source rows of centers_rep are the center
    replicated 4x).  Descriptor generation (~9ns/idx/worker, 4 workers) is a
    hard wall, so K divides it.  Partial groups are completed with pad rows
    whose x equals the class center (contributing ~0), and filler groups point
    at a zeros row.
  * x moves as fp8-e4m3 (half the bytes) and is upconverted to bf16 by the
    otherwise-idle ACT engine while the Q7 SWDGE library load (~9-12us, the
    single largest fixed cost) is still in flight; compute then runs at the
    bf16 2x DVE tier instead of the 1x fp8 tier.
  * the scalar reduction tail is clipped: per-partition accumulators go to the
    host directly (2KB), no on-device cross-partition matmul.

Sharding: x/labels split into 8 contiguous row shards; centers replicated.
"""

import numpy as np
import ml_dtypes

import concourse.bacc as bacc
import concourse.tile as tile
from concourse import mybir
from concourse.bass_utils import run_bass_kernel_spmd
from concourse.library_config import mlp

N, C, D = 65536, 1000, 128
N_CORES = 8
P = 128
ROWS_PER_CORE = N // N_CORES            # 8192

# --- tunables -------------------------------------------------------------
DTYPE_X = "fp8"         # dtype of x in DRAM ("bf16" | "fp8")
DTYPE_C = "bf16"        # dtype of gathered centers + compute
K = 4                   # rows per gather index (host sorts + pads classes)
NCHUNK = 4              # compute/gather chunks (chunk rows must be % 128*K)
X_DMAS = 2              # number of x dma_start instructions
SINGLE_PACKET = False
QUEUES = [1, 2, 3, 0]   # SWDGE queue per chunk

_NP_DT = {"bf16": ml_dtypes.bfloat16, "fp8": ml_dtypes.float8_e4m3fn}
_MY_DT = {"bf16": mybir.dt.bfloat16, "fp8": mybir.dt.float8e4}

_NC_CACHE = {}


def _plan(labels_np):
    """Padded-rows plan shared by all cores (single SPMD program)."""
    if K == 1:
        rows_p = ROWS_PER_CORE
    else:
        need = 0
        for m in range(N_CORES):
            cnt = np.bincount(labels_np[m * ROWS_PER_CORE:(m + 1) * ROWS_PER_CORE],
                              minlength=C)
            need = max(need, ROWS_PER_CORE + int(((-cnt) % K).sum()))
        rows_p = -(-need // (NCHUNK * P * K)) * (NCHUNK * P * K)
    chunk = rows_p // NCHUNK
    return rows_p, chunk


def _build_nc(rows_p, chunk):
    f32 = mybir.dt.float32
    dtx = _MY_DT[DTYPE_X]
    dtc = _MY_DT[DTYPE_C]
    gc = chunk // K                 # gather indices per chunk
    s2 = gc // P                    # group slots per partition per chunk
    cols = chunk                    # elements per partition per chunk
    icols = gc // 16                # idx columns per chunk

    nc = bacc.Bacc(trn_type="TRN2", num_swdge_queues=4, dynamic_dma_scratch_size=65536)

    x = nc.dram_tensor("x", [NCHUNK, P, cols], dtx, kind="ExternalInput")
    idx16 = nc.dram_tensor("idx16", [P, NCHUNK * icols], mybir.dt.int16,
                           kind="ExternalInput")
    centers_rep = nc.dram_tensor("centers_rep", [C + 1, K * D], dtc,
                                 kind="ExternalInput")
    out = nc.dram_tensor("out", [P, NCHUNK], f32, kind="ExternalOutput")

    with tile.TileContext(nc) as tc:
        # first Pool-queue instruction: the SWDGE/extended-inst library load
        # (MODIFY_POOL_CONFIG + ~9-12us of IRAM DMA) gates all gathers
        nc.gpsimd.load_library(mlp)
        with (
            tc.tile_pool(name="cp", bufs=NCHUNK) as cp,
            tc.tile_pool(name="small", bufs=1) as small,
        ):
            idx = small.tile([P, NCHUNK * icols], mybir.dt.int16)
            nc.sync.dma_start(out=idx[:], in_=idx16.ap())

            acc = small.tile([P, NCHUNK], f32)
            cts = [cp.tile([P, cols], dtc, tag="ct", name=f"ct{c}")
                   for c in range(NCHUNK)]
            for c in range(NCHUNK):
                nc.gpsimd.dma_gather(
                    cts[c][:].rearrange("p (s e) -> p s e", s=s2),
                    centers_rep.ap(),
                    idx[:, c * icols:(c + 1) * icols],
                    gc, gc, K * D,
                    queue_num=QUEUES[c],
                    single_packet=SINGLE_PACKET,
                )

            # x loads: a few large DMAs (each dma_start costs ~600ns of Sync)
            xts = [None] * NCHUNK
            per = NCHUNK // X_DMAS
            for g in range(X_DMAS):
                dst = small.tile([P, per * cols], dtx, tag=f"xg{g}")
                nc.sync.dma_start(
                    out=dst[:].rearrange("p (c f) -> p c f", c=per),
                    in_=x.ap()[g * per:(g + 1) * per].rearrange("c p f -> p c f"),
                )
                for c in range(g * per, (g + 1) * per):
                    xts[c] = dst[:, (c - g * per) * cols:(c - g * per + 1) * cols]

            # fp8 -> bf16 upconvert on ACT while the library load is in flight
            if DTYPE_X != DTYPE_C:
                xbs = [small.tile([P, cols], dtc, name=f"xb{c}")
                       for c in range(NCHUNK)]
                for c in range(NCHUNK):
                    nc.scalar.activation(out=xbs[c][:], in_=xts[c],
                                         func=mybir.ActivationFunctionType.Copy)
                xts = [xb[:] for xb in xbs]

            for c in range(NCHUNK):
                xt, ct = xts[c], cts[c]
                nc.vector.tensor_tensor(out=xt, in0=xt, in1=ct[:],
                                        op=mybir.AluOpType.subtract)
                nc.scalar.activation(
                    out=xt, in_=xt,
                    func=mybir.ActivationFunctionType.Square,
                    accum_out=acc[:, c:c + 1],
                )

            nc.sync.dma_start(out=out.ap(), in_=acc[:])

    nc.compile()
    return nc


def _get_nc(rows_p, chunk):
    key = (DTYPE_X, DTYPE_C, K, NCHUNK, X_DMAS, SINGLE_PACKET, tuple(QUEUES), rows_p)
    if key not in _NC_CACHE:
        _NC_CACHE[key] = _build_nc(rows_p, chunk)
    return _NC_CACHE[key]


def _core_inputs(x_core, lab_core, centers, rows_p, chunk):
    """Sort rows by label, pad class segments to K, lay out x in device order
    and build the wrapped int16 gather indices.  Pad slots inside a partial
    group are filled with that class's center so (x_pad - c)^2 ~= 0; filler
    groups beyond the last class point at the zeros row of centers_rep."""
    np_dt = _NP_DT[DTYPE_X]
    gc = chunk // K
    s2 = gc // P
    icols = gc // 16
    total_g = rows_p // K

    order = np.argsort(lab_core, kind="stable")
    slab = lab_core[order]
    vals = x_core[order]
    if K == 1:
        g_of_row = np.arange(ROWS_PER_CORE)
        g_class = slab.astype(np.int16)
        n_groups = ROWS_PER_CORE
    else:
        cnt = np.bincount(slab, minlength=C)
        pad = (-cnt) % K
        gcnt = (cnt + pad) // K
        n_groups = int(gcnt.sum())
        g_class_real = np.repeat(np.arange(C), gcnt).astype(np.int16)
        gbase = np.concatenate([[0], np.cumsum(gcnt)[:-1]])
        within = np.arange(ROWS_PER_CORE) - np.repeat(
            np.concatenate([[0], np.cumsum(cnt)[:-1]]), cnt)
        g_of_row = gbase[slab] * K + within
        g_class = np.full(total_g, C, dtype=np.int16)
        g_class[:n_groups] = g_class_real
        # pad slots of partial groups: x := c_j so the pad contributes ~0
        pj = np.repeat(np.arange(C), pad)
        poff = np.arange(int(pad.sum())) - np.repeat(
            np.concatenate([[0], np.cumsum(pad)[:-1]]), pad)
        g_of_row = np.concatenate([g_of_row, gbase[pj] * K + cnt[pj] + poff])
        vals = np.concatenate([vals, centers[pj]])

    # device x layout: group g -> chunk g//gc, partition g%P, slot g//P (the
    # gathered index i lands on partition i%128, slot i//128)
    x_dev = np.zeros((NCHUNK, P, s2, K, D), dtype=np_dt)
    grp = g_of_row // K
    k_off = g_of_row % K
    c_i = grp // gc
    g_loc = grp % gc
    x_dev[c_i, g_loc % P, g_loc // P, k_off, :] = vals.astype(np_dt)

    idx16 = np.zeros((16, NCHUNK * icols), dtype=np.int16)
    gg = np.arange(total_g)
    idx16[(gg % gc) % 16, (gg // gc) * icols + (gg % gc) // 16] = g_class
    return (x_dev.reshape(NCHUNK, P, chunk),
            np.ascontiguousarray(np.tile(idx16, (8, 1))))


def make_in_maps(x, labels, centers, rows_p, chunk):
    x = np.ascontiguousarray(np.asarray(x), dtype=np.float32)
    labels_np = np.asarray(labels).astype(np.int64)
    centers = np.asarray(centers).astype(np.float32)
    crep = np.zeros((C + 1, K * D), dtype=_NP_DT[DTYPE_C])
    crep[:C] = np.tile(centers, (1, K)).astype(_NP_DT[DTYPE_C])
    in_maps = []
    for m in range(N_CORES):
        lo = m * ROWS_PER_CORE
        x_dev, idx16 = _core_inputs(x[lo:lo + ROWS_PER_CORE],
                                    labels_np[lo:lo + ROWS_PER_CORE], centers,
                                    rows_p, chunk)
        in_maps.append({"x": x_dev, "idx16": idx16, "centers_rep": crep})
    return in_maps


def run(x, labels, centers, **spmd_kwargs):
    """Run on the 8 NeuronCores; returns (loss, BassKernelResults)."""
    labels_np = np.asarray(labels).astype(np.int64)
    rows_p, chunk = _plan(labels_np)
    nc = _get_nc(rows_p, chunk)
    in_maps = make_in_maps(x, labels_np, centers, rows_p, chunk)
    res = run_bass_kernel_spmd(nc, in_maps, core_ids=list(range(N_CORES)), **spmd_kwargs)
    total = sum(float(r["out"].astype(np.float64).sum()) for r in res.results)
    return np.float32(total / N), res


def kernel(x, labels, centers):
    loss, _ = run(x, labels, centers)
    return loss
